# revision 1
# baseline (speedup 1.0000x reference)
"""MoE (8 experts, top-2) Trainium2 kernel — expert-parallel across 8 NeuronCores.

Strategy (per the expert-parallel sharding hint):
  - Routing (gate matmul + top-2 + softmax) runs on host: it is tiny
    (4096x1024 @ 1024x8) and produces the router_logits output directly.
  - Dispatch: tokens are gathered per expert on host (all-to-all equivalent),
    padded to a fixed capacity CAP, and each NeuronCore runs ONE expert's
    dense FFN (x @ w1.T -> gelu -> @ w2.T) over its gathered tokens in bf16
    with fp32 PSUM accumulation.
  - Combine: host scatter-adds each expert's outputs scaled by the softmax
    weight (combine matrix) back into the full [T, H] output.

Device kernel layout (per core, SPMD identical program):
  xt  [128, 8, CAP]  bf16  : x gathered+transposed, h = po*128 + pi
  w1t [128, 8, 4096] bf16  : w1[e].T  (lhsT tiles for mm1)
  w2t [128, 32,1024] bf16  : w2[e].T  (lhsT tiles for mm2)
  yt  [128, 8, CAP]  fp32  : y.T output, o = oc*128 + pi
"""

import numpy as np
import ml_dtypes

NUM_EXPERTS = 8
TOP_K = 2
HIDDEN = 1024
INTER = 4096
N_CORES = 8

# Per-expert token capacity per launch. Expected per-expert load is
# T*K/E = 1024; CAP covers typical imbalance. Overflow is handled with
# additional launches, so any distribution is correct.
CAP = 1152
CHUNK = 384
NCHUNK = CAP // CHUNK

BF16 = ml_dtypes.bfloat16

_NC_CACHE = None


def _build_nc():
    import concourse.mybir as mybir
    import concourse.tile as tile
    from concourse import bacc

    dt = mybir.dt
    nc = bacc.Bacc("TRN2", target_bir_lowering=False, debug=False)
    xt = nc.dram_tensor("xt", [128, 8, CAP], dt.bfloat16, kind="ExternalInput")
    w1t = nc.dram_tensor("w1t", [128, 8, INTER], dt.bfloat16, kind="ExternalInput")
    w2t = nc.dram_tensor("w2t", [128, 32, HIDDEN], dt.bfloat16, kind="ExternalInput")
    yt = nc.dram_tensor("yt", [128, 8, CAP], dt.float32, kind="ExternalOutput")

    with tile.TileContext(nc) as tc:
        with (
            tc.tile_pool(name="wpool", bufs=1) as wpool,
            tc.tile_pool(name="xpool", bufs=1) as xpool,
            tc.tile_pool(name="hpool", bufs=1) as hpool,
            tc.tile_pool(name="opool", bufs=3) as opool,
            tc.tile_pool(name="ps1", bufs=4, space="PSUM") as ps1,
            tc.tile_pool(name="ps2", bufs=3, space="PSUM") as ps2,
        ):
            x_sb = xpool.tile([128, 8, CAP], dt.bfloat16)
            nc.sync.dma_start(x_sb[:], xt.ap()[:])
            w1_sb = wpool.tile([128, 8, INTER], dt.bfloat16, tag="w1")
            # split weight DMAs so early matmul groups can start before the
            # whole weight tensor lands
            for g in range(8):
                sl = slice(g * 512, (g + 1) * 512)
                nc.sync.dma_start(w1_sb[:, :, sl], w1t.ap()[:, :, sl])
            w2_sb = wpool.tile([128, 32, HIDDEN], dt.bfloat16, tag="w2")
            for g in range(8):
                sl = slice(g * 128, (g + 1) * 128)
                nc.sync.dma_start(w2_sb[:, :, sl], w2t.ap()[:, :, sl])

            for c in range(NCHUNK):
                tsl = slice(c * CHUNK, (c + 1) * CHUNK)
                h_sb = hpool.tile([128, 32, CHUNK], dt.bfloat16, tag="h")
                # mm1: h.T[i, t] = sum_h w1T[h, i] * x.T[h, t], then gelu
                for m in range(32):
                    ps = ps1.tile([128, CHUNK], dt.float32, tag="ps1")
                    msl = slice(m * 128, (m + 1) * 128)
                    for k in range(8):
                        nc.tensor.matmul(
                            ps[:],
                            w1_sb[:, k, msl],
                            x_sb[:, k, tsl],
                            start=(k == 0),
                            stop=(k == 7),
                        )
                    nc.scalar.activation(
                        h_sb[:, m, :], ps[:], mybir.ActivationFunctionType.Gelu
                    )
                # mm2: y.T[o, t] = sum_i w2T[i, o] * h.T[i, t]
                for o in range(8):
                    ps_o = ps2.tile([128, CHUNK], dt.float32, tag="ps2")
                    osl = slice(o * 128, (o + 1) * 128)
                    for i in range(32):
                        nc.tensor.matmul(
                            ps_o[:],
                            w2_sb[:, i, osl],
                            h_sb[:, i, :],
                            start=(i == 0),
                            stop=(i == 31),
                        )
                    o_sb = opool.tile([128, CHUNK], dt.float32, tag="o")
                    nc.vector.tensor_copy(o_sb[:], ps_o[:])
                    nc.sync.dma_start(yt.ap()[:, o, tsl], o_sb[:])
    nc.compile()
    return nc


def _get_nc():
    global _NC_CACHE
    if _NC_CACHE is None:
        _NC_CACHE = _build_nc()
    return _NC_CACHE


def _pack_x(x_rows):
    """[n<=CAP, HIDDEN] fp32 -> [128, 8, CAP] bf16 (h = po*128 + pi)."""
    n = x_rows.shape[0]
    xg = np.zeros((CAP, HIDDEN), dtype=BF16)
    xg[:n] = x_rows.astype(BF16)
    return np.ascontiguousarray(xg.T.reshape(8, 128, CAP).transpose(1, 0, 2))


def _pack_w1(w1_e):
    """[INTER, HIDDEN] -> [128, 8, INTER] bf16 (w1.T tiled over h)."""
    return np.ascontiguousarray(
        w1_e.T.astype(BF16).reshape(8, 128, INTER).transpose(1, 0, 2)
    )


def _pack_w2(w2_e):
    """[HIDDEN, INTER] -> [128, 32, HIDDEN] bf16 (w2.T tiled over i)."""
    return np.ascontiguousarray(
        w2_e.T.astype(BF16).reshape(32, 128, HIDDEN).transpose(1, 0, 2)
    )


def kernel(hidden_states, gate_w, w1, w2):
    from concourse.bass_utils import run_bass_kernel_spmd

    B, S, H = hidden_states.shape
    x = np.asarray(hidden_states, dtype=np.float32).reshape(-1, H)
    T = x.shape[0]

    # ---- routing on host (tiny) ----
    router_logits = (x @ np.asarray(gate_w, dtype=np.float32).T).astype(np.float32)
    top_idx = np.argsort(-router_logits, axis=1)[:, :TOP_K]
    top_vals = np.take_along_axis(router_logits, top_idx, axis=1)
    mx = top_vals.max(axis=1, keepdims=True)
    ex = np.exp(top_vals - mx)
    router_weights = (ex / ex.sum(axis=1, keepdims=True)).astype(np.float32)

    # ---- dispatch: jobs of (expert, token rows, combine scale), <= CAP each ----
    w1 = np.asarray(w1)
    w2 = np.asarray(w2)
    jobs = []
    for e in range(NUM_EXPERTS):
        mask = top_idx == e
        rows = np.nonzero(mask.any(axis=1))[0]
        if rows.size == 0:
            continue
        scale = np.where(mask[rows, 0], router_weights[rows, 0], router_weights[rows, 1])
        for s0 in range(0, rows.size, CAP):
            jobs.append((e, rows[s0 : s0 + CAP], scale[s0 : s0 + CAP]))

    nc = _get_nc()
    w1_packed = {}
    w2_packed = {}
    out = np.zeros((T, H), dtype=np.float32)

    for b0 in range(0, len(jobs), N_CORES):
        batch = jobs[b0 : b0 + N_CORES]
        in_maps = []
        for e, rows, _ in batch:
            if e not in w1_packed:
                w1_packed[e] = _pack_w1(w1[e])
                w2_packed[e] = _pack_w2(w2[e])
            in_maps.append(
                {"xt": _pack_x(x[rows]), "w1t": w1_packed[e], "w2t": w2_packed[e]}
            )
        while len(in_maps) < N_CORES:  # idle cores on a ragged last batch
            in_maps.append(
                {k: np.zeros_like(v) for k, v in in_maps[0].items()}
            )
        res = run_bass_kernel_spmd(nc, in_maps, core_ids=list(range(N_CORES)))
        for j, (e, rows, scale) in enumerate(batch):
            ytj = res.results[j]["yt"]  # [128, 8, CAP]
            y = ytj.transpose(2, 1, 0).reshape(CAP, H)[: rows.size]
            out[rows] += scale[:, None] * y

    return out.reshape(B, S, H), router_logits
